# revision 8
# baseline (speedup 1.0000x reference)
"""BertSelfAttention Trainium2 Bass kernel (v6).

B=8, S=1024, D=1024, H=16 heads, head_dim=64. Data-parallel: batch element b
runs on NeuronCore b (no collectives).

Numerics: all matmuls run single-pass fp16 (inputs rounded to fp16 on the
host, products accumulated in fp32 PSUM). Rel err ~1e-3 vs the fp32
reference, inside the 2e-2 gate. Host-side input marshalling: X shipped as
X^T fp16; Wq/Wk shipped fp16 packed by column-chunk [c, p, k, m] so head
pair c's stationary weights arrive as one contiguous DMA; Wv shipped fp16
row-chunked (it streams as the moving operand).

Per-core schedule, software-pipelined across head pairs c (heads 2c, 2c+1
live in d-chunk c of Q^T/K^T):
  Q^T_c = Wq^T X^T + bq, K^T_c = Wk^T X^T + bk  (bias in the DVE evacuation)
  scoresT[k, q] per head on PE row-tiles (0,0)/(64,0) -- the two 64-row K=64
    matmuls stream concurrently at full array rate -- into per-head 2-bank
    PSUM tiles (bufs=2) so ACT drains head A while PE fills head B
  expT = exp(scoresT/8 + mask[k])  (ACT, N=1024 per instr, fp16 out)
  ctx[q, 66] = sum_k expT[k, q]^T [V_h|1][k, :]  direct form: expT chunks are
    the stationary operand (FWL fp16), no ctx transpose; the ones column
    accumulates the softmax denominator in the same PSUM tile
  normalize with per-partition reciprocal multiply straight PSUM->SBUF.

The V projection (V = X Wv + bv via a K=1 ones-row matmul; adding bv to V
before the normalized A*V equals ctx+bv after) and the previous pair's A*V
groups are emitted BETWEEN scores steps, so the in-order PE queue always has
dense work while ACT drains exp:
  pair0: scores only | pair1: + V cols 0-511 | av_0 block |
  pair2: + av_1 (odd) + V cols 512-1023 (even, first half) |
  pair3: + av_2 (odd) + V cols 512-1023 (even, second half) |
  pair4..7: + av_{c-1} (odd) | av_7 tail.
"""

import sys

sys.path.insert(0, "/opt/trn_rl_repo")

import numpy as np

import concourse.bass as bass  # noqa: E402
import concourse.tile as tile  # noqa: E402
from concourse import bacc, mybir  # noqa: E402
from concourse.bass import ds, ts  # noqa: E402
from concourse.bass_utils import run_bass_kernel_spmd  # noqa: E402

B, S, D, H = 8, 1024, 1024, 16
HD = D // H  # 64
P = 128
NCH = S // P  # 8
HP = HD + 2  # 66: head block incl. ones column (+pad to keep 8B alignment)
FP32 = mybir.dt.float32
FP16 = mybir.dt.float16
EXP = mybir.ActivationFunctionType.Exp

_CACHED = {}


def _build_kernel(tc):
    nc = tc.nc
    xt_d = nc.dram_tensor("xt", [D, S], FP16, kind="ExternalInput").ap()
    mask_d = nc.dram_tensor("mask", [S], FP32, kind="ExternalInput").ap()
    # Wq/Wk column-packed: [c, p, k*128+m] = W[128k+p, 128c+m]
    wq_d = nc.dram_tensor("Wq", [NCH, P, D], FP16, kind="ExternalInput").ap()
    bq_d = nc.dram_tensor("bq", [D], FP32, kind="ExternalInput").ap()
    wk_d = nc.dram_tensor("Wk", [NCH, P, D], FP16, kind="ExternalInput").ap()
    bk_d = nc.dram_tensor("bk", [D], FP32, kind="ExternalInput").ap()
    wv_d = nc.dram_tensor("Wv", [D, D], FP16, kind="ExternalInput").ap()
    bv_d = nc.dram_tensor("bv", [D], FP32, kind="ExternalInput").ap()
    out_d = nc.dram_tensor("out", [S, D], FP32, kind="ExternalOutput").ap()

    mm = nc.tensor.matmul

    with (
        tc.tile_pool(name="const", bufs=1) as const,
        tc.tile_pool(name="persist", bufs=1) as persist,
    ):
        mask_sb = const.tile([P, NCH], FP32)
        bq_sb = const.tile([P, NCH], FP32)
        bk_sb = const.tile([P, NCH], FP32)
        bv_sb = const.tile([1, D], FP32)
        bv16 = const.tile([1, D], FP16)
        ones_row = const.tile([1, P], FP16)

        xt = persist.tile([P, NCH, S], FP16, tag="xt")  # X^T: [f, s]
        v_sb = persist.tile([P, NCH, H, HP], FP16, tag="v")  # V+bv: [k, h, d|1]
        wq16 = persist.tile([P, NCH, NCH, P], FP16, tag="wq16")  # [p, c, k, m]
        wk16 = persist.tile([P, NCH, NCH, P], FP16, tag="wk16")
        wv16 = persist.tile([P, NCH, D], FP16, tag="wv16")

        # DMA plan: first W column-chunk ahead of X^T so the first projection
        # can trickle-start as X^T chunks land; Wv + consts on the gpsimd
        # queue; remaining W columns follow in pair order.
        nc.sync.dma_start(out=wq16[:, 0], in_=wq_d[0])
        nc.scalar.dma_start(out=wk16[:, 0], in_=wk_d[0])
        for j in range(0, NCH, 2):
            nc.sync.dma_start(out=xt[:, j], in_=xt_d[ts(j, P), :])
            nc.scalar.dma_start(out=xt[:, j + 1], in_=xt_d[ts(j + 1, P), :])
        for c in range(1, NCH):
            nc.sync.dma_start(out=wq16[:, c], in_=wq_d[c])
            nc.scalar.dma_start(out=wk16[:, c], in_=wk_d[c])
        nc.gpsimd.dma_start(out=mask_sb[:], in_=mask_d.rearrange("(c p) -> p c", p=P))
        nc.gpsimd.dma_start(out=bq_sb[:], in_=bq_d.rearrange("(c p) -> p c", p=P))
        nc.gpsimd.dma_start(out=bk_sb[:], in_=bk_d.rearrange("(c p) -> p c", p=P))
        nc.gpsimd.dma_start(out=bv_sb[:], in_=bv_d.rearrange("(a d) -> a d", a=1))
        for k in range(NCH):
            nc.gpsimd.dma_start(out=wv16[:, k], in_=wv_d[ts(k, P), :])
        nc.vector.tensor_copy(out=bv16[:], in_=bv_sb[:])
        nc.gpsimd.memset(ones_row[:], 1.0)
        # ones columns accumulate the softmax denominator during A*V
        nc.gpsimd.memset(v_sb[:, :, :, HD:HP], 1.0)

        with (
            tc.tile_pool(name="ppsum", bufs=2, space="PSUM") as ppsum,
            tc.tile_pool(name="spsum", bufs=1, space="PSUM") as spsum,
            tc.tile_pool(name="avpsum", bufs=2, space="PSUM") as avpsum,
            tc.tile_pool(name="exppool", bufs=4) as exppool,
            tc.tile_pool(name="qkpool", bufs=2) as qkpool,
            tc.tile_pool(name="obpool", bufs=3) as obpool,
            tc.tile_pool(name="rnpool", bufs=8) as rnpool,
        ):
            exp_tiles = {}
            ob_tiles = {}

            def emit_qk_proj(c):
                tiles = []
                for w16, b_sb, tag in ((wq16, bq_sb, "qt"), (wk16, bk_sb, "kt")):
                    dst = qkpool.tile([P, S], FP16, tag=tag, name=f"{tag}{c}")
                    tiles.append(dst)
                    for n in range(2):
                        po = ppsum.tile([P, 512], FP32, tag="proj")
                        for k in range(NCH):
                            mm(po[:], w16[:, c, k, :], xt[:, k, ts(n, 512)],
                               start=(k == 0), stop=(k == NCH - 1))
                        nc.vector.tensor_scalar_add(
                            dst[:, ts(n, 512)], po[:], b_sb[:, c : c + 1]
                        )
                return tiles

            def emit_scores_step(c, qtc, ktc, i):
                for hh in range(2):
                    if i == 0:
                        exp_tiles[2 * c + hh] = exppool.tile(
                            [P, NCH, S], FP16, tag="exp", name=f"exp{2 * c + hh}"
                        )
                # one 4-bank tile for both heads: the two 64-row matmuls on PE
                # row-tiles (0,0)/(64,0) stream concurrently
                sp = spsum.tile([P, 2, S], FP32, tag="scores", name=f"sp{c}_{i}")
                for n in range(2):
                    for hh in range(2):
                        oh = HD * hh
                        mm(sp[:, hh, ts(n, 512)],
                           ktc[oh : oh + HD, ts(i, P)],
                           qtc[oh : oh + HD, ts(n, 512)],
                           start=True, stop=True)
                for hh in range(2):
                    nc.scalar.activation(
                        out=exp_tiles[2 * c + hh][:, i, :],
                        in_=sp[:, hh, :],
                        func=EXP,
                        bias=mask_sb[:, i : i + 1],
                        scale=1.0 / np.sqrt(HD).item(),
                    )

            def emit_av_group(c, hh, g):
                """One (head, q-half) A*V group: 32 MMs + normalize; DMA the
                head out after its second group."""
                h = 2 * c + hh
                exp_h = exp_tiles[h]
                if g == 0:
                    ob_tiles[h] = obpool.tile(
                        [P, NCH, HD], FP32, tag="ob", name=f"ob{h}"
                    )
                ob = ob_tiles[h]
                avp = avpsum.tile([P, 4, HP], FP32, tag="av")
                for cq in range(4):
                    q0 = 4 * g + cq
                    for k in range(NCH):
                        mm(avp[:, cq, :],
                           exp_h[:, k, ts(q0, P)],
                           v_sb[:, k, h, :],
                           start=(k == 0), stop=(k == NCH - 1))
                rn = rnpool.tile([P, 4], FP32, tag="rn")
                nc.vector.reciprocal(
                    rn[:], avp[:, :, HD : HD + 1].rearrange("p a b -> p (a b)")
                )
                for cq in range(4):
                    nc.vector.tensor_scalar_mul(
                        ob[:, 4 * g + cq, :], avp[:, cq, 0:HD], rn[:, cq : cq + 1]
                    )
                if g == 1:
                    del exp_tiles[h]
                    nc.sync.dma_start(
                        out=out_d[:, ds(HD * h, HD)].rearrange(
                            "(j p) d -> p j d", p=P
                        ),
                        in_=ob_tiles.pop(h)[:],
                    )

            def emit_v_group(c, n):
                po = ppsum.tile([P, 512], FP32, tag="proj", name=f"vp{c}_{n}")
                for k in range(NCH):
                    mm(po[:], xt[:, k, ts(c, P)], wv16[:, k, ts(n, 512)],
                       start=(k == 0), stop=False)
                mm(po[:], ones_row[:], bv16[:, ts(n, 512)], start=False, stop=True)
                nc.vector.tensor_copy(
                    out=v_sb[:, c, ds(8 * n, 8), 0:HD],
                    in_=po.rearrange("p (h d) -> p h d", d=HD),
                )

            AV4 = ((0, 0), (0, 1), (1, 0), (1, 1))

            # --- pipeline (see module docstring) ---
            q0t, k0t = emit_qk_proj(0)
            for i in range(NCH):
                emit_scores_step(0, q0t, k0t, i)
            q1t, k1t = emit_qk_proj(1)
            for i in range(NCH):
                emit_scores_step(1, q1t, k1t, i)
                emit_v_group(i, 0)
            for hh, g in AV4:
                emit_av_group(0, hh, g)
            for c in range(2, NCH):
                qtc, ktc = emit_qk_proj(c)
                for i in range(NCH):
                    emit_scores_step(c, qtc, ktc, i)
                    if i % 2 == 1:
                        hh, g = AV4[i // 2]
                        emit_av_group(c - 1, hh, g)
                    elif c in (2, 3):
                        emit_v_group(4 * (c - 2) + i // 2, 1)
            for hh, g in AV4:
                emit_av_group(NCH - 1, hh, g)


def _ensure_ntff_hook():
    """antenv.axon_hooks is absent in this image; recreate it so
    run_bass_kernel_spmd(trace=True) can capture NTFF profiles."""
    import types

    try:
        from antenv.axon_hooks import get_axon_ntff_profile_hook  # noqa: F401

        return
    except ImportError:
        pass
    from trn_agent_boot.trn_boot import _ntff_profile_via_ctypes

    hook = _ntff_profile_via_ctypes("/opt/axon/libaxon_pjrt.so")
    mod = types.ModuleType("antenv.axon_hooks")
    mod._hook = hook
    mod.get_axon_ntff_profile_hook = lambda: mod._hook
    mod.set_axon_ntff_profile_hook = lambda h: setattr(mod, "_hook", h)
    sys.modules["antenv.axon_hooks"] = mod


def _get_compiled():
    if "nc" not in _CACHED:
        nc = bacc.Bacc(
            "TRN2", target_bir_lowering=False, debug=False, num_devices=B
        )
        with tile.TileContext(nc) as tc:
            _build_kernel(tc)
        nc.compile()
        _CACHED["nc"] = nc
    return _CACHED["nc"]


def _pack_cols(w16):
    # [c, p, k*128+m] = W[128k+p, 128c+m]
    return np.ascontiguousarray(
        w16.reshape(NCH, P, NCH, P).transpose(2, 1, 0, 3).reshape(NCH, P, D)
    )


def kernel(hidden_states, attention_mask, Wq, bq, Wk, bk, Wv, bv, **run_kwargs):
    hs16 = np.asarray(hidden_states, dtype=np.float32).astype(np.float16)
    xts = [np.ascontiguousarray(hs16[b].T) for b in range(B)]
    am = np.ascontiguousarray(np.asarray(attention_mask, dtype=np.float32)).reshape(B, S)
    weights = {
        "Wq": _pack_cols(np.asarray(Wq, dtype=np.float32).astype(np.float16)),
        "bq": np.ascontiguousarray(np.asarray(bq, dtype=np.float32)),
        "Wk": _pack_cols(np.asarray(Wk, dtype=np.float32).astype(np.float16)),
        "bk": np.ascontiguousarray(np.asarray(bk, dtype=np.float32)),
        "Wv": np.ascontiguousarray(np.asarray(Wv, dtype=np.float32).astype(np.float16)),
        "bv": np.ascontiguousarray(np.asarray(bv, dtype=np.float32)),
    }
    if run_kwargs.get("trace"):
        _ensure_ntff_hook()
    nc = _get_compiled()
    in_maps = [
        {"xt": xts[b], "mask": am[b], **weights} for b in range(B)
    ]
    res = run_bass_kernel_spmd(nc, in_maps, core_ids=list(range(B)), **run_kwargs)
    out = np.stack([res.results[b]["out"] for b in range(B)], axis=0)
    if run_kwargs:
        kernel.last_results = res
    return out


if __name__ == "__main__":
    rng = np.random.default_rng(0)
    inputs = {
        "hidden_states": rng.standard_normal((B, S, D), dtype=np.float32),
        "attention_mask": np.zeros((B, 1, 1, S), dtype=np.float32),
        "Wq": rng.standard_normal((D, D), dtype=np.float32) / 32.0,
        "bq": rng.standard_normal(D, dtype=np.float32) * 0.02,
        "Wk": rng.standard_normal((D, D), dtype=np.float32) / 32.0,
        "bk": rng.standard_normal(D, dtype=np.float32) * 0.02,
        "Wv": rng.standard_normal((D, D), dtype=np.float32) / 32.0,
        "bv": rng.standard_normal(D, dtype=np.float32) * 0.02,
    }
    out = kernel(**inputs)
    print("out", out.shape, out.dtype, float(np.abs(out).mean()))


# revision 10
# speedup vs baseline: 1.1583x; 1.1583x over previous
"""BertSelfAttention Trainium2 Bass kernel (v6).

B=8, S=1024, D=1024, H=16 heads, head_dim=64. Data-parallel: batch element b
runs on NeuronCore b (no collectives).

Numerics: all matmuls run single-pass fp16 (inputs rounded to fp16 on the
host, products accumulated in fp32 PSUM). Rel err ~1e-3 vs the fp32
reference, inside the 2e-2 gate. Host-side input marshalling: X shipped as
X^T fp16; Wq/Wk shipped fp16 packed by column-chunk [c, p, k, m] so head
pair c's stationary weights arrive as one contiguous DMA; Wv shipped fp16
row-chunked (it streams as the moving operand).

Per-core schedule, software-pipelined across head pairs c (heads 2c, 2c+1
live in d-chunk c of Q^T/K^T):
  Q^T_c = Wq^T X^T + bq, K^T_c = Wk^T X^T + bk  (bias in the DVE evacuation)
  scoresT[k, q] per head on PE row-tiles (0,0)/(64,0) -- the two 64-row K=64
    matmuls stream concurrently at full array rate -- into per-head 2-bank
    PSUM tiles (bufs=2) so ACT drains head A while PE fills head B
  expT = exp(scoresT/8 + mask[k])  (ACT, N=1024 per instr, fp16 out)
  ctx[q, 66] = sum_k expT[k, q]^T [V_h|1][k, :]  direct form: expT chunks are
    the stationary operand (FWL fp16), no ctx transpose; the ones column
    accumulates the softmax denominator in the same PSUM tile
  normalize with per-partition reciprocal multiply straight PSUM->SBUF.

The V projection (V = X Wv + bv via a K=1 ones-row matmul; adding bv to V
before the normalized A*V equals ctx+bv after) and the previous pair's A*V
groups are emitted BETWEEN scores steps, so the in-order PE queue always has
dense work while ACT drains exp:
  pair0: scores only | pair1: + V cols 0-511 | av_0 block |
  pair2: + av_1 (odd) + V cols 512-1023 (even, first half) |
  pair3: + av_2 (odd) + V cols 512-1023 (even, second half) |
  pair4..7: + av_{c-1} (odd) | av_7 tail.
"""

import sys

sys.path.insert(0, "/opt/trn_rl_repo")

import numpy as np

import concourse.bass as bass  # noqa: E402
import concourse.tile as tile  # noqa: E402
from concourse import bacc, mybir  # noqa: E402
from concourse.bass import ds, ts  # noqa: E402
from concourse.bass_utils import run_bass_kernel_spmd  # noqa: E402

B, S, D, H = 8, 1024, 1024, 16
HD = D // H  # 64
P = 128
NCH = S // P  # 8
HP = HD + 2  # 66: head block incl. ones column (+pad to keep 8B alignment)
FP32 = mybir.dt.float32
FP16 = mybir.dt.float16
EXP = mybir.ActivationFunctionType.Exp

_CACHED = {}


def _build_kernel(tc):
    nc = tc.nc
    xt_d = nc.dram_tensor("xt", [D, S], FP16, kind="ExternalInput").ap()
    mask_d = nc.dram_tensor("mask", [S], FP32, kind="ExternalInput").ap()
    # Wq/Wk column-packed: [c, p, k*128+m] = W[128k+p, 128c+m]
    wq_d = nc.dram_tensor("Wq", [NCH, P, D], FP16, kind="ExternalInput").ap()
    bq_d = nc.dram_tensor("bq", [D], FP32, kind="ExternalInput").ap()
    wk_d = nc.dram_tensor("Wk", [NCH, P, D], FP16, kind="ExternalInput").ap()
    bk_d = nc.dram_tensor("bk", [D], FP32, kind="ExternalInput").ap()
    wv_d = nc.dram_tensor("Wv", [D, D], FP16, kind="ExternalInput").ap()
    bv_d = nc.dram_tensor("bv", [D], FP32, kind="ExternalInput").ap()
    out_d = nc.dram_tensor("out", [S, D], FP32, kind="ExternalOutput").ap()

    mm = nc.tensor.matmul

    with (
        tc.tile_pool(name="const", bufs=1) as const,
        tc.tile_pool(name="persist", bufs=1) as persist,
    ):
        mask_sb = const.tile([P, NCH], FP32)
        bq_sb = const.tile([P, NCH], FP32)
        bk_sb = const.tile([P, NCH], FP32)
        bv_sb = const.tile([1, D], FP32)
        bv16 = const.tile([1, D], FP16)
        ones_row = const.tile([1, P], FP16)

        xt = persist.tile([P, NCH, S], FP16, tag="xt")  # X^T: [f, s]
        v_sb = persist.tile([P, NCH, H, HP], FP16, tag="v")  # V+bv: [k, h, d|1]
        wq16 = persist.tile([P, NCH, NCH, P], FP16, tag="wq16")  # [p, c, k, m]
        wk16 = persist.tile([P, NCH, NCH, P], FP16, tag="wk16")
        wv16 = persist.tile([P, NCH, D], FP16, tag="wv16")

        # DMA plan: first W column-chunk ahead of X^T so the first projection
        # can trickle-start as X^T chunks land; Wv + consts on the gpsimd
        # queue; remaining W columns follow in pair order.
        nc.sync.dma_start(out=wq16[:, 0], in_=wq_d[0])
        nc.scalar.dma_start(out=wk16[:, 0], in_=wk_d[0])
        for j in range(0, NCH, 2):
            nc.sync.dma_start(out=xt[:, j], in_=xt_d[ts(j, P), :])
            nc.scalar.dma_start(out=xt[:, j + 1], in_=xt_d[ts(j + 1, P), :])
        for c in range(1, NCH):
            nc.sync.dma_start(out=wq16[:, c], in_=wq_d[c])
            nc.scalar.dma_start(out=wk16[:, c], in_=wk_d[c])
        nc.gpsimd.dma_start(out=mask_sb[:], in_=mask_d.rearrange("(c p) -> p c", p=P))
        nc.gpsimd.dma_start(out=bq_sb[:], in_=bq_d.rearrange("(c p) -> p c", p=P))
        nc.gpsimd.dma_start(out=bk_sb[:], in_=bk_d.rearrange("(c p) -> p c", p=P))
        nc.gpsimd.dma_start(out=bv_sb[:], in_=bv_d.rearrange("(a d) -> a d", a=1))
        for k in range(NCH):
            nc.gpsimd.dma_start(out=wv16[:, k], in_=wv_d[ts(k, P), :])
        nc.vector.tensor_copy(out=bv16[:], in_=bv_sb[:])
        nc.gpsimd.memset(ones_row[:], 1.0)
        # ones columns accumulate the softmax denominator during A*V
        nc.gpsimd.memset(v_sb[:, :, :, HD:HP], 1.0)

        with (
            tc.tile_pool(name="ppsum", bufs=2, space="PSUM") as ppsum,
            tc.tile_pool(name="spsum", bufs=2, space="PSUM") as spsum,
            tc.tile_pool(name="avpsum", bufs=2, space="PSUM") as avpsum,
            tc.tile_pool(name="exppool", bufs=4) as exppool,
            tc.tile_pool(name="qkpool", bufs=2) as qkpool,
            tc.tile_pool(name="obpool", bufs=3) as obpool,
            tc.tile_pool(name="rnpool", bufs=8) as rnpool,
        ):
            exp_tiles = {}
            ob_tiles = {}

            def emit_qk_proj(c):
                tiles = []
                for w16, b_sb, tag in ((wq16, bq_sb, "qt"), (wk16, bk_sb, "kt")):
                    dst = qkpool.tile([P, S], FP16, tag=tag, name=f"{tag}{c}")
                    tiles.append(dst)
                    for n in range(2):
                        po = ppsum.tile([P, 512], FP32, tag="proj")
                        for k in range(NCH):
                            mm(po[:], w16[:, c, k, :], xt[:, k, ts(n, 512)],
                               start=(k == 0), stop=(k == NCH - 1))
                        nc.vector.tensor_scalar_add(
                            dst[:, ts(n, 512)], po[:], b_sb[:, c : c + 1]
                        )
                return tiles

            def emit_scores_step(c, qtc, ktc, i):
                for hh in range(2):
                    if i == 0:
                        exp_tiles[2 * c + hh] = exppool.tile(
                            [P, NCH, S], FP16, tag="exp", name=f"exp{2 * c + hh}"
                        )
                # both heads share a 2-bank tile so their 64-row matmuls run
                # concurrently on PE row-tiles (0,0)/(64,0); bufs=2 lets ACT
                # drain step (i,n) while PE fills (i,n+1)
                for n in range(2):
                    sp = spsum.tile(
                        [P, 2, 512], FP32, tag="scores", name=f"sp{c}_{i}_{n}"
                    )
                    for hh in range(2):
                        oh = HD * hh
                        mm(sp[:, hh, :],
                           ktc[oh : oh + HD, ts(i, P)],
                           qtc[oh : oh + HD, ts(n, 512)],
                           start=True, stop=True)
                    for hh in range(2):
                        nc.scalar.activation(
                            out=exp_tiles[2 * c + hh][:, i, ts(n, 512)],
                            in_=sp[:, hh, :],
                            func=EXP,
                            bias=mask_sb[:, i : i + 1],
                            scale=1.0 / np.sqrt(HD).item(),
                        )

            def emit_av_group(c, hh, g):
                """One (head, q-half) A*V group: 32 MMs + normalize; DMA the
                head out after its second group."""
                h = 2 * c + hh
                exp_h = exp_tiles[h]
                if g == 0:
                    ob_tiles[h] = obpool.tile(
                        [P, NCH, HD], FP32, tag="ob", name=f"ob{h}"
                    )
                ob = ob_tiles[h]
                avp = avpsum.tile([P, 4, HP], FP32, tag="av")
                for cq in range(4):
                    q0 = 4 * g + cq
                    for k in range(NCH):
                        mm(avp[:, cq, :],
                           exp_h[:, k, ts(q0, P)],
                           v_sb[:, k, h, :],
                           start=(k == 0), stop=(k == NCH - 1))
                rn = rnpool.tile([P, 4], FP32, tag="rn")
                nc.vector.reciprocal(
                    rn[:], avp[:, :, HD : HD + 1].rearrange("p a b -> p (a b)")
                )
                for cq in range(4):
                    nc.vector.tensor_scalar_mul(
                        ob[:, 4 * g + cq, :], avp[:, cq, 0:HD], rn[:, cq : cq + 1]
                    )
                if g == 1:
                    del exp_tiles[h]
                    nc.sync.dma_start(
                        out=out_d[:, ds(HD * h, HD)].rearrange(
                            "(j p) d -> p j d", p=P
                        ),
                        in_=ob_tiles.pop(h)[:],
                    )

            def emit_v_group(c, n):
                po = ppsum.tile([P, 512], FP32, tag="proj", name=f"vp{c}_{n}")
                for k in range(NCH):
                    mm(po[:], xt[:, k, ts(c, P)], wv16[:, k, ts(n, 512)],
                       start=(k == 0), stop=False)
                mm(po[:], ones_row[:], bv16[:, ts(n, 512)], start=False, stop=True)
                nc.vector.tensor_copy(
                    out=v_sb[:, c, ds(8 * n, 8), 0:HD],
                    in_=po.rearrange("p (h d) -> p h d", d=HD),
                )

            AV4 = ((0, 0), (0, 1), (1, 0), (1, 1))

            # --- pipeline (see module docstring) ---
            q0t, k0t = emit_qk_proj(0)
            for i in range(NCH):
                emit_scores_step(0, q0t, k0t, i)
            q1t, k1t = emit_qk_proj(1)
            for i in range(NCH):
                emit_scores_step(1, q1t, k1t, i)
                emit_v_group(i, 0)
            for hh, g in AV4:
                emit_av_group(0, hh, g)
            for c in range(2, NCH):
                qtc, ktc = emit_qk_proj(c)
                for i in range(NCH):
                    emit_scores_step(c, qtc, ktc, i)
                    if i % 2 == 1:
                        hh, g = AV4[i // 2]
                        emit_av_group(c - 1, hh, g)
                    elif c in (2, 3):
                        emit_v_group(4 * (c - 2) + i // 2, 1)
            for hh, g in AV4:
                emit_av_group(NCH - 1, hh, g)


def _ensure_ntff_hook():
    """antenv.axon_hooks is absent in this image; recreate it so
    run_bass_kernel_spmd(trace=True) can capture NTFF profiles."""
    import types

    try:
        from antenv.axon_hooks import get_axon_ntff_profile_hook  # noqa: F401

        return
    except ImportError:
        pass
    from trn_agent_boot.trn_boot import _ntff_profile_via_ctypes

    hook = _ntff_profile_via_ctypes("/opt/axon/libaxon_pjrt.so")
    mod = types.ModuleType("antenv.axon_hooks")
    mod._hook = hook
    mod.get_axon_ntff_profile_hook = lambda: mod._hook
    mod.set_axon_ntff_profile_hook = lambda h: setattr(mod, "_hook", h)
    sys.modules["antenv.axon_hooks"] = mod


def _get_compiled():
    if "nc" not in _CACHED:
        nc = bacc.Bacc(
            "TRN2", target_bir_lowering=False, debug=False, num_devices=B
        )
        with tile.TileContext(nc) as tc:
            _build_kernel(tc)
        nc.compile()
        _CACHED["nc"] = nc
    return _CACHED["nc"]


def _pack_cols(w16):
    # [c, p, k*128+m] = W[128k+p, 128c+m]
    return np.ascontiguousarray(
        w16.reshape(NCH, P, NCH, P).transpose(2, 1, 0, 3).reshape(NCH, P, D)
    )


def kernel(hidden_states, attention_mask, Wq, bq, Wk, bk, Wv, bv, **run_kwargs):
    hs16 = np.asarray(hidden_states, dtype=np.float32).astype(np.float16)
    xts = [np.ascontiguousarray(hs16[b].T) for b in range(B)]
    am = np.ascontiguousarray(np.asarray(attention_mask, dtype=np.float32)).reshape(B, S)
    weights = {
        "Wq": _pack_cols(np.asarray(Wq, dtype=np.float32).astype(np.float16)),
        "bq": np.ascontiguousarray(np.asarray(bq, dtype=np.float32)),
        "Wk": _pack_cols(np.asarray(Wk, dtype=np.float32).astype(np.float16)),
        "bk": np.ascontiguousarray(np.asarray(bk, dtype=np.float32)),
        "Wv": np.ascontiguousarray(np.asarray(Wv, dtype=np.float32).astype(np.float16)),
        "bv": np.ascontiguousarray(np.asarray(bv, dtype=np.float32)),
    }
    if run_kwargs.get("trace"):
        _ensure_ntff_hook()
    nc = _get_compiled()
    in_maps = [
        {"xt": xts[b], "mask": am[b], **weights} for b in range(B)
    ]
    res = run_bass_kernel_spmd(nc, in_maps, core_ids=list(range(B)), **run_kwargs)
    out = np.stack([res.results[b]["out"] for b in range(B)], axis=0)
    if run_kwargs:
        kernel.last_results = res
    return out


if __name__ == "__main__":
    rng = np.random.default_rng(0)
    inputs = {
        "hidden_states": rng.standard_normal((B, S, D), dtype=np.float32),
        "attention_mask": np.zeros((B, 1, 1, S), dtype=np.float32),
        "Wq": rng.standard_normal((D, D), dtype=np.float32) / 32.0,
        "bq": rng.standard_normal(D, dtype=np.float32) * 0.02,
        "Wk": rng.standard_normal((D, D), dtype=np.float32) / 32.0,
        "bk": rng.standard_normal(D, dtype=np.float32) * 0.02,
        "Wv": rng.standard_normal((D, D), dtype=np.float32) / 32.0,
        "bv": rng.standard_normal(D, dtype=np.float32) * 0.02,
    }
    out = kernel(**inputs)
    print("out", out.shape, out.dtype, float(np.abs(out).mean()))


# revision 13
# speedup vs baseline: 1.1809x; 1.0195x over previous
"""BertSelfAttention Trainium2 Bass kernel (v6).

B=8, S=1024, D=1024, H=16 heads, head_dim=64. Data-parallel: batch element b
runs on NeuronCore b (no collectives).

Numerics: all matmuls run single-pass fp16 (inputs rounded to fp16 on the
host, products accumulated in fp32 PSUM). Rel err ~1e-3 vs the fp32
reference, inside the 2e-2 gate. Host-side input marshalling: X shipped as
X^T fp16; Wq/Wk shipped fp16 packed by column-chunk [c, p, k, m] so head
pair c's stationary weights arrive as one contiguous DMA; Wv shipped fp16
row-chunked (it streams as the moving operand).

Per-core schedule, software-pipelined across head pairs c (heads 2c, 2c+1
live in d-chunk c of Q^T/K^T):
  Q^T_c = Wq^T X^T + bq, K^T_c = Wk^T X^T + bk  (bias in the DVE evacuation)
  scoresT[k, q] per head on PE row-tiles (0,0)/(64,0) -- the two 64-row K=64
    matmuls stream concurrently at full array rate -- into per-head 2-bank
    PSUM tiles (bufs=2) so ACT drains head A while PE fills head B
  expT = exp(scoresT/8 + mask[k])  (ACT, N=1024 per instr, fp16 out)
  ctx[q, 66] = sum_k expT[k, q]^T [V_h|1][k, :]  direct form: expT chunks are
    the stationary operand (FWL fp16), no ctx transpose; the ones column
    accumulates the softmax denominator in the same PSUM tile
  normalize with per-partition reciprocal multiply straight PSUM->SBUF.

The V projection (V = X Wv + bv via a K=1 ones-row matmul; adding bv to V
before the normalized A*V equals ctx+bv after) and the previous pair's A*V
groups are emitted BETWEEN scores steps, so the in-order PE queue always has
dense work while ACT drains exp:
  pair0: scores only | pair1: + V cols 0-511 | av_0 block |
  pair2: + av_1 (odd) + V cols 512-1023 (even, first half) |
  pair3: + av_2 (odd) + V cols 512-1023 (even, second half) |
  pair4..7: + av_{c-1} (odd) | av_7 tail.
"""

import sys
from collections import deque

sys.path.insert(0, "/opt/trn_rl_repo")

import numpy as np

import concourse.bass as bass  # noqa: E402
import concourse.tile as tile  # noqa: E402
from concourse import bacc, mybir  # noqa: E402
from concourse.bass import ds, ts  # noqa: E402
from concourse.bass_utils import run_bass_kernel_spmd  # noqa: E402

B, S, D, H = 8, 1024, 1024, 16
HD = D // H  # 64
P = 128
NCH = S // P  # 8
HP = HD + 2  # 66: head block incl. ones column (+pad to keep 8B alignment)
FP32 = mybir.dt.float32
FP16 = mybir.dt.float16
EXP = mybir.ActivationFunctionType.Exp

_CACHED = {}


def _build_kernel(tc):
    nc = tc.nc
    xt_d = nc.dram_tensor("xt", [D, S], FP16, kind="ExternalInput").ap()
    mask_d = nc.dram_tensor("mask", [S], FP32, kind="ExternalInput").ap()
    # Wq/Wk column-packed: [c, p, k*128+m] = W[128k+p, 128c+m]
    wq_d = nc.dram_tensor("Wq", [NCH, P, D], FP16, kind="ExternalInput").ap()
    bq_d = nc.dram_tensor("bq", [D], FP32, kind="ExternalInput").ap()
    wk_d = nc.dram_tensor("Wk", [NCH, P, D], FP16, kind="ExternalInput").ap()
    bk_d = nc.dram_tensor("bk", [D], FP32, kind="ExternalInput").ap()
    wv_d = nc.dram_tensor("Wv", [D, D], FP16, kind="ExternalInput").ap()
    bv_d = nc.dram_tensor("bv", [D], FP32, kind="ExternalInput").ap()
    out_d = nc.dram_tensor("out", [S, D], FP32, kind="ExternalOutput").ap()

    mm = nc.tensor.matmul

    with (
        tc.tile_pool(name="const", bufs=1) as const,
        tc.tile_pool(name="persist", bufs=1) as persist,
    ):
        mask_sb = const.tile([P, NCH], FP32)
        bq_sb = const.tile([P, NCH], FP32)
        bk_sb = const.tile([P, NCH], FP32)
        bv_sb = const.tile([1, D], FP32)
        bv16 = const.tile([1, D], FP16)
        ones_row = const.tile([1, P], FP16)

        xt = persist.tile([P, NCH, S], FP16, tag="xt")  # X^T: [f, s]
        v_sb = persist.tile([P, NCH, H, HP], FP16, tag="v")  # V+bv: [k, h, d|1]
        wq16 = persist.tile([P, NCH, NCH, P], FP16, tag="wq16")  # [p, c, k, m]
        wk16 = persist.tile([P, NCH, NCH, P], FP16, tag="wk16")
        wv16 = persist.tile([P, NCH, D], FP16, tag="wv16")

        # DMA plan: first W column-chunk ahead of X^T so the first projection
        # can trickle-start as X^T chunks land; Wv + consts on the gpsimd
        # queue; remaining W columns follow in pair order.
        nc.sync.dma_start(out=wq16[:, 0], in_=wq_d[0])
        nc.scalar.dma_start(out=wk16[:, 0], in_=wk_d[0])
        for j in range(0, NCH, 2):
            nc.sync.dma_start(out=xt[:, j], in_=xt_d[ts(j, P), :])
            nc.scalar.dma_start(out=xt[:, j + 1], in_=xt_d[ts(j + 1, P), :])
        for c in range(1, NCH):
            nc.sync.dma_start(out=wq16[:, c], in_=wq_d[c])
            nc.scalar.dma_start(out=wk16[:, c], in_=wk_d[c])
        nc.gpsimd.dma_start(out=mask_sb[:], in_=mask_d.rearrange("(c p) -> p c", p=P))
        nc.gpsimd.dma_start(out=bq_sb[:], in_=bq_d.rearrange("(c p) -> p c", p=P))
        nc.gpsimd.dma_start(out=bk_sb[:], in_=bk_d.rearrange("(c p) -> p c", p=P))
        nc.gpsimd.dma_start(out=bv_sb[:], in_=bv_d.rearrange("(a d) -> a d", a=1))
        for k in range(NCH):
            nc.gpsimd.dma_start(out=wv16[:, k], in_=wv_d[ts(k, P), :])
        nc.vector.tensor_copy(out=bv16[:], in_=bv_sb[:])
        nc.gpsimd.memset(ones_row[:], 1.0)
        # ones columns accumulate the softmax denominator during A*V
        nc.gpsimd.memset(v_sb[:, :, :, HD:HP], 1.0)

        with (
            tc.tile_pool(name="ppsum", bufs=2, space="PSUM") as ppsum,
            tc.tile_pool(name="spsum", bufs=2, space="PSUM") as spsum,
            tc.tile_pool(name="avpsum", bufs=2, space="PSUM") as avpsum,
            tc.tile_pool(name="exppool", bufs=4) as exppool,
            tc.tile_pool(name="qkpool", bufs=2) as qkpool,
            tc.tile_pool(name="obpool", bufs=3) as obpool,
            tc.tile_pool(name="rnpool", bufs=8) as rnpool,
        ):
            exp_tiles = {}
            ob_tiles = {}
            qk_tiles = {}
            filler = deque()

            def pop_fillers(slots_left):
                k = (len(filler) + slots_left - 1) // slots_left
                for _ in range(min(k, len(filler))):
                    filler.popleft()()

            def qk_piece(c, which, n):
                """One quarter of a Q^T/K^T projection: 8 accumulating MMs +
                biased DVE evacuation."""
                def go():
                    w16, b_sb, tag = (
                        (wq16, bq_sb, "qt"), (wk16, bk_sb, "kt")
                    )[which]
                    key = (tag, c)
                    if key not in qk_tiles:
                        qk_tiles[key] = qkpool.tile(
                            [P, S], FP16, tag=tag, name=f"{tag}{c}"
                        )
                    dst = qk_tiles[key]
                    po = ppsum.tile([P, 512], FP32, tag="proj")
                    for k in range(NCH):
                        mm(po[:], w16[:, c, k, :], xt[:, k, ts(n, 512)],
                           start=(k == 0), stop=(k == NCH - 1))
                    nc.vector.tensor_scalar_add(
                        dst[:, ts(n, 512)], po[:], b_sb[:, c : c + 1]
                    )
                return go

            def emit_qk_proj(c):
                for which in range(2):
                    for n in range(2):
                        qk_piece(c, which, n)()
                return qk_tiles[("qt", c)], qk_tiles[("kt", c)]

            def emit_scores_step(c, qtc, ktc, i):
                for hh in range(2):
                    if i == 0:
                        exp_tiles[2 * c + hh] = exppool.tile(
                            [P, NCH, S], FP16, tag="exp", name=f"exp{2 * c + hh}"
                        )
                # both heads share a 2-bank tile so their 64-row matmuls run
                # concurrently on PE row-tiles (0,0)/(64,0); bufs=2 lets ACT
                # drain step (i,n) while PE fills (i,n+1)
                for n in range(2):
                    sp = spsum.tile(
                        [P, 2, 512], FP32, tag="scores", name=f"sp{c}_{i}_{n}"
                    )
                    for hh in range(2):
                        oh = HD * hh
                        mm(sp[:, hh, :],
                           ktc[oh : oh + HD, ts(i, P)],
                           qtc[oh : oh + HD, ts(n, 512)],
                           start=True, stop=True)
                    for hh in range(2):
                        nc.scalar.activation(
                            out=exp_tiles[2 * c + hh][:, i, ts(n, 512)],
                            in_=sp[:, hh, :],
                            func=EXP,
                            bias=mask_sb[:, i : i + 1],
                            scale=1.0 / np.sqrt(HD).item(),
                        )

            def emit_av_group(c, hh, g):
                """One (head, q-half) A*V group: 32 MMs + normalize; DMA the
                head out after its second group."""
                h = 2 * c + hh
                exp_h = exp_tiles[h]
                if g == 0:
                    ob_tiles[h] = obpool.tile(
                        [P, NCH, HD], FP32, tag="ob", name=f"ob{h}"
                    )
                ob = ob_tiles[h]
                avp = avpsum.tile([P, 4, HP], FP32, tag="av")
                for cq in range(4):
                    q0 = 4 * g + cq
                    for k in range(NCH):
                        mm(avp[:, cq, :],
                           exp_h[:, k, ts(q0, P)],
                           v_sb[:, k, h, :],
                           start=(k == 0), stop=(k == NCH - 1))
                rn = rnpool.tile([P, 4], FP32, tag="rn")
                nc.vector.reciprocal(
                    rn[:], avp[:, :, HD : HD + 1].rearrange("p a b -> p (a b)")
                )
                for cq in range(4):
                    nc.vector.tensor_scalar_mul(
                        ob[:, 4 * g + cq, :], avp[:, cq, 0:HD], rn[:, cq : cq + 1]
                    )
                if g == 1:
                    del exp_tiles[h]
                    nc.sync.dma_start(
                        out=out_d[:, ds(HD * h, HD)].rearrange(
                            "(j p) d -> p j d", p=P
                        ),
                        in_=ob_tiles.pop(h)[:],
                    )

            def emit_v_group(c, n):
                po = ppsum.tile([P, 512], FP32, tag="proj", name=f"vp{c}_{n}")
                for k in range(NCH):
                    mm(po[:], xt[:, k, ts(c, P)], wv16[:, k, ts(n, 512)],
                       start=(k == 0), stop=False)
                mm(po[:], ones_row[:], bv16[:, ts(n, 512)], start=False, stop=True)
                nc.vector.tensor_copy(
                    out=v_sb[:, c, ds(8 * n, 8), 0:HD],
                    in_=po.rearrange("p (h d) -> p h d", d=HD),
                )

            AV4 = ((0, 0), (0, 1), (1, 0), (1, 1))

            def av_filler(c):
                return [
                    (lambda hh=hh, g=g: emit_av_group(c, hh, g))
                    for hh, g in AV4
                ]

            def interleave(*lists):
                out = []
                n = max(len(l) for l in lists)
                for i in range(n):
                    for l in lists:
                        if i < len(l):
                            out.append(l[i])
                return out

            # --- pipeline: every scores slot pops PE filler work (next
            # pair's projection pieces, V-projection pieces, previous pair's
            # A*V groups) so PE stays dense while ACT drains exp ---
            q0t, k0t = emit_qk_proj(0)
            filler.extend(qk_piece(1, w, n) for w in range(2) for n in range(2))
            for i in range(NCH):
                emit_scores_step(0, q0t, k0t, i)
                pop_fillers(NCH - i)
            q1t, k1t = qk_tiles[("qt", 1)], qk_tiles[("kt", 1)]
            filler.extend(interleave(
                [lambda cc=cc: emit_v_group(cc, 0) for cc in range(NCH)],
                [qk_piece(2, w, n) for w in range(2) for n in range(2)],
            ))
            for i in range(NCH):
                emit_scores_step(1, q1t, k1t, i)
                pop_fillers(NCH - i)
            for hh, g in AV4:
                emit_av_group(0, hh, g)
            for c in range(2, NCH):
                qtc, ktc = qk_tiles[("qt", c)], qk_tiles[("kt", c)]
                others = []
                if c < NCH - 1:
                    others.extend(qk_piece(c + 1, w, n)
                                  for w in range(2) for n in range(2))
                if c in (2, 3):
                    others.extend(
                        lambda cc=cc: emit_v_group(cc, 1)
                        for cc in range(4 * (c - 2), 4 * (c - 1))
                    )
                filler.extend(interleave(av_filler(c - 1), others))
                for i in range(NCH):
                    emit_scores_step(c, qtc, ktc, i)
                    pop_fillers(NCH - i)
            for hh, g in AV4:
                emit_av_group(NCH - 1, hh, g)


def _ensure_ntff_hook():
    """antenv.axon_hooks is absent in this image; recreate it so
    run_bass_kernel_spmd(trace=True) can capture NTFF profiles."""
    import types

    try:
        from antenv.axon_hooks import get_axon_ntff_profile_hook  # noqa: F401

        return
    except ImportError:
        pass
    from trn_agent_boot.trn_boot import _ntff_profile_via_ctypes

    hook = _ntff_profile_via_ctypes("/opt/axon/libaxon_pjrt.so")
    mod = types.ModuleType("antenv.axon_hooks")
    mod._hook = hook
    mod.get_axon_ntff_profile_hook = lambda: mod._hook
    mod.set_axon_ntff_profile_hook = lambda h: setattr(mod, "_hook", h)
    sys.modules["antenv.axon_hooks"] = mod


def _get_compiled():
    if "nc" not in _CACHED:
        nc = bacc.Bacc(
            "TRN2", target_bir_lowering=False, debug=False, num_devices=B
        )
        with tile.TileContext(nc) as tc:
            _build_kernel(tc)
        nc.compile()
        _CACHED["nc"] = nc
    return _CACHED["nc"]


def _pack_cols(w16):
    # [c, p, k*128+m] = W[128k+p, 128c+m]
    return np.ascontiguousarray(
        w16.reshape(NCH, P, NCH, P).transpose(2, 1, 0, 3).reshape(NCH, P, D)
    )


def kernel(hidden_states, attention_mask, Wq, bq, Wk, bk, Wv, bv, **run_kwargs):
    hs16 = np.asarray(hidden_states, dtype=np.float32).astype(np.float16)
    xts = [np.ascontiguousarray(hs16[b].T) for b in range(B)]
    am = np.ascontiguousarray(np.asarray(attention_mask, dtype=np.float32)).reshape(B, S)
    weights = {
        "Wq": _pack_cols(np.asarray(Wq, dtype=np.float32).astype(np.float16)),
        "bq": np.ascontiguousarray(np.asarray(bq, dtype=np.float32)),
        "Wk": _pack_cols(np.asarray(Wk, dtype=np.float32).astype(np.float16)),
        "bk": np.ascontiguousarray(np.asarray(bk, dtype=np.float32)),
        "Wv": np.ascontiguousarray(np.asarray(Wv, dtype=np.float32).astype(np.float16)),
        "bv": np.ascontiguousarray(np.asarray(bv, dtype=np.float32)),
    }
    if run_kwargs.get("trace"):
        _ensure_ntff_hook()
    nc = _get_compiled()
    in_maps = [
        {"xt": xts[b], "mask": am[b], **weights} for b in range(B)
    ]
    res = run_bass_kernel_spmd(nc, in_maps, core_ids=list(range(B)), **run_kwargs)
    out = np.stack([res.results[b]["out"] for b in range(B)], axis=0)
    if run_kwargs:
        kernel.last_results = res
    return out


if __name__ == "__main__":
    rng = np.random.default_rng(0)
    inputs = {
        "hidden_states": rng.standard_normal((B, S, D), dtype=np.float32),
        "attention_mask": np.zeros((B, 1, 1, S), dtype=np.float32),
        "Wq": rng.standard_normal((D, D), dtype=np.float32) / 32.0,
        "bq": rng.standard_normal(D, dtype=np.float32) * 0.02,
        "Wk": rng.standard_normal((D, D), dtype=np.float32) / 32.0,
        "bk": rng.standard_normal(D, dtype=np.float32) * 0.02,
        "Wv": rng.standard_normal((D, D), dtype=np.float32) / 32.0,
        "bv": rng.standard_normal(D, dtype=np.float32) * 0.02,
    }
    out = kernel(**inputs)
    print("out", out.shape, out.dtype, float(np.abs(out).mean()))
